# revision 24
# baseline (speedup 1.0000x reference)
"""BiModalAttention Trainium2 kernel.

Math (per batch b):
    S = x @ y.T                      [4096, 4096]
    a1 = softmax_rows(S)  @ y * x    [4096, 64]
    a2 = softmax_rows(S.T) @ x * y   [4096, 64]
    out = concat(a1, a2, axis=-1)    [4096, 128]

Sharding: data-parallel over batch, one batch per NeuronCore (8 cores).

Per-core algorithm (one direction; the other swaps x and y):
  The logits S never touch HBM.  Because |S| <= ~45 for randn inputs,
  exp(S) stays inside fp32 range, so softmax needs no max-subtraction:
  softmax(S)_st = exp(S_st) / sum_t exp(S_st).

  Pass over S^T tiles [t=128, s-chunk]:
    ST = yT_slice.T @ xT_chunk          (PE, f32r)
    E  = exp(ST)                        (ACT, writes f32r)
    acc[s-chunk] += [y | 1].T @ E       (PE, accumulate over all t)
  acc is [65, s-chunk]: rows 0..63 = (softmax_rows(S) @ y).T unnormalized,
  row 64 = the softmax row sums.  Finalize: PE-transpose acc, multiply by
  x and by reciprocal(row sums) per-partition, DMA to out.
"""

import numpy as np

import concourse.bass as bass
import concourse.mybir as mybir
import concourse.tile as tile
from concourse import bacc
from concourse.bass_utils import run_bass_kernel_spmd

B = 8
S = 4096
D = 64
P = 128
NT = S // P     # 32 row tiles
CH = 512        # psum chunk (one bank of fp32)
NCH = S // CH   # 8 chunks
HALF = NCH // 2  # 4 chunks per half-pass

F32 = mybir.dt.float32
F32R = mybir.dt.float32r
BF16 = mybir.dt.bfloat16
FP16 = mybir.dt.float16
EXP = mybir.ActivationFunctionType.Exp

# exp output / PV operand dtype.  The ACT engine is output-write-bound for
# fp32 (~1.36 ns/col measured); bf16 halves the write bytes and runs at
# ~0.57 ns/col.  PV matmul in bf16 is the same 1 cycle/row as f32r, and
# bf16's ~0.4% relative error on the (positive) exp values is well inside
# the 2e-2 tolerance.
ET_DT = BF16

# QK logits matmul dtype.  FP16 keeps f32r-level precision (10 mantissa
# bits, same as tf32; inputs are N(0,1) so range is no concern) while its
# 2-byte weights background-load through the PE's ldweights path instead
# of serializing ~210ns per matmul like 4-byte f32r weights do.
QK_DT = FP16

_CACHE = {}

# --- experiment knobs (module-level so harnesses can poke them) ---
QK_SLOTS = 2    # psum ring slots for qk tiles ([128, GRP*CH] = 2 banks each)
QK_TP = False   # emit QK as 2 concurrent 64x64 tile-position half-matmuls
PV_MERGE = False  # single 1024-col PV matmul instead of 2x 512
QK_WDT = None   # unused (fp16 QK path supersedes the bf16-weights idea)


def _build(repeat: int = 1, mode: str = "full", loop: int = 1):
    nc = bacc.Bacc()
    x = nc.declare_dram_parameter("x", [S, D], F32, isOutput=False)
    y = nc.declare_dram_parameter("y", [S, D], F32, isOutput=False)
    xT = nc.declare_dram_parameter("xT", [D, S], QK_DT, isOutput=False)
    yT = nc.declare_dram_parameter("yT", [D, S], QK_DT, isOutput=False)
    xTf = nc.declare_dram_parameter("xTf", [D, S], F32, isOutput=False)
    yTf = nc.declare_dram_parameter("yTf", [D, S], F32, isOutput=False)
    ident = nc.declare_dram_parameter("ident", [P, P], F32, isOutput=False)
    out = nc.declare_dram_parameter("out", [S, 2 * D], F32, isOutput=True)

    with tile.TileContext(nc) as tc:
        with (
            tc.tile_pool(name="singles", bufs=1) as singles,
            tc.tile_pool(name="et", bufs=6) as etp,
            tc.tile_pool(name="fin", bufs=2) as fin,
            tc.tile_pool(name="psum", bufs=2, space="PSUM") as psum,
        ):
            # ---- load inputs ----
            # f32r matmul operands straight from DRAM (host passes the same
            # fp32 bytes; hardware rounds/interprets f32r on read).  The
            # first compute needs yT tile 0 and xT chunks 0..1, so the loads
            # are split with the critical pieces first on the sync HWDGE;
            # everything not needed immediately goes via gpsimd SWDGE.
            xT_r = singles.tile([D, S], QK_DT)
            yT_r = singles.tile([D, S], QK_DT)
            Q1 = 1024
            nc.sync.dma_start(out=xT_r[:, 0:Q1], in_=xT[:, 0:Q1])
            nc.sync.dma_start(out=yT_r[:, 0:Q1], in_=yT[:, 0:Q1])
            nc.sync.dma_start(out=yT_r[:, Q1:S], in_=yT[:, Q1:S])
            nc.sync.dma_start(out=xT_r[:, Q1:S], in_=xT[:, Q1:S])
            x_sb = singles.tile([P, NT, D], F32)
            y_sb = singles.tile([P, NT, D], F32)
            y_re = y[:, :].rearrange("(n p) d -> p n d", p=P)
            x_re = x[:, :].rearrange("(n p) d -> p n d", p=P)
            NT1 = 8
            nc.gpsimd.dma_start(out=y_sb[:, 0:NT1], in_=y_re[:, 0:NT1])
            nc.gpsimd.dma_start(out=x_sb[:, 0:NT1], in_=x_re[:, 0:NT1])
            nc.gpsimd.dma_start(out=y_sb[:, NT1:NT], in_=y_re[:, NT1:NT])
            nc.gpsimd.dma_start(out=x_sb[:, NT1:NT], in_=x_re[:, NT1:NT])
            # fp32 transposed copies for the finalize elementwise multiply
            xT_f = singles.tile([D, S], F32)
            yT_f = singles.tile([D, S], F32)
            nc.gpsimd.dma_start(out=xT_f, in_=xTf[:, :])
            nc.gpsimd.dma_start(out=yT_f, in_=yTf[:, :])
            id_sb = singles.tile([P, P], F32)
            nc.gpsimd.dma_start(out=id_sb, in_=ident[:, :])

            # Duplicated-across-halves operands for tile-position QK halves
            if QK_TP:
                xT2 = singles.tile([P, S], QK_DT)
                yT2 = singles.tile([P, S], QK_DT)
                nc.gpsimd.dma_start(out=xT2[0:D, :], in_=xT[:, :])
                nc.gpsimd.dma_start(out=xT2[D : 2 * D, :], in_=xT[:, :])
                nc.gpsimd.dma_start(out=yT2[0:D, :], in_=yT[:, :])
                nc.gpsimd.dma_start(out=yT2[D : 2 * D, :], in_=yT[:, :])

            # [V | 1] stationary operands for the PV matmul
            vp_r = singles.tile([P, NT, D + 1], ET_DT)
            xp_r = singles.tile([P, NT, D + 1], ET_DT)
            ones_f = singles.tile([P, NT, 1], F32)
            nc.vector.memset(ones_f, 1.0)
            if mode == "pvonly":
                etc = singles.tile([P, 2 * CH], ET_DT)
                nc.vector.memset(etc, 0.25)
            nc.vector.tensor_copy(out=vp_r[:, :, D : D + 1], in_=ones_f)
            nc.vector.tensor_copy(out=xp_r[:, :, D : D + 1], in_=ones_f)
            nc.vector.tensor_copy(out=vp_r[:, 0:NT1, 0:D], in_=y_sb[:, 0:NT1])
            nc.vector.tensor_copy(out=xp_r[:, 0:NT1, 0:D], in_=x_sb[:, 0:NT1])
            nc.vector.tensor_copy(out=vp_r[:, NT1:NT, 0:D], in_=y_sb[:, NT1:NT])
            nc.vector.tensor_copy(out=xp_r[:, NT1:NT, 0:D], in_=x_sb[:, NT1:NT])

            # The s axis is processed in groups of GRP chunks (one qk tile /
            # exp covers a whole group).  With GRP=2 the two live PSUM
            # accumulators of a group plus the two of the previous group fit
            # in 4 banks, so a new group's PV matmuls never wait on finalize.
            # Finalize work for a completed group is interleaved into the
            # NEXT group's loop: the oacc->sbuf copies are emitted up front
            # (freeing the accumulators), the transpose+scale+store of each
            # chunk spreads over the first iterations.  trp tiles borrow the
            # frequently-recycled "qk" slots (sharing "oacc" would deadlock
            # against the live accumulators of the current group).
            GRP = 2
            NGRP = NCH // GRP

            def emit_finalize_head(pending):
                osbs = []
                for cl in range(GRP):
                    c = pending["g"] * GRP + cl
                    osb = fin.tile([D + 1, CH], F32, tag="osb", bufs=4,
                                   name=f"osb_{pending['direction']}_{pending['g']}_{cl}")
                    nc.vector.tensor_copy(out=osb, in_=pending["oaccs"][cl])
                    osbs.append(osb)
                for cl in range(GRP):
                    c = pending["g"] * GRP + cl
                    nc.vector.tensor_mul(
                        osbs[cl][0:D, :], osbs[cl][0:D, :],
                        pending["qTf"][:, c * CH : (c + 1) * CH],
                    )
                    nc.vector.reciprocal(
                        out=osbs[cl][D : D + 1, :], in_=osbs[cl][D : D + 1, :]
                    )
                pending["osbs"] = osbs

            def emit_finalize_chunk(pending, cl):
                c = pending["g"] * GRP + cl
                osb = pending["osbs"][cl]
                for k in range(CH // P):
                    st = c * (CH // P) + k  # s-tile index
                    # "oacc" slots: 2 are always free under GRP=2 (released
                    # by the head copies), so these never stall the QK
                    # pipeline and never cycle-wait against live accumulators.
                    trp = psum.tile([P, D + 1], F32, tag="oacc", bufs=4)
                    nc.tensor.transpose(
                        trp,
                        osb[:, k * P : (k + 1) * P],
                        id_sb[: D + 1, : D + 1],
                    )
                    a_t = fin.tile([P, D], F32, tag="a", bufs=4)
                    nc.vector.tensor_scalar_mul(
                        a_t, trp[:, 0:D], trp[:, D : D + 1]
                    )
                    nc.sync.dma_start(
                        out=pending["out_ap"][st * P : (st + 1) * P],
                        in_=a_t,
                    )

            def emit_repeat():
              pending = None
              for direction in range(2):
                qT = xT_r if direction == 0 else yT_r   # moving operand (s axis)
                kT = yT_r if direction == 0 else xT_r   # stationary (t axis)
                if QK_TP:
                    qT2 = xT2 if direction == 0 else yT2
                    kT2 = yT2 if direction == 0 else xT2
                vp = vp_r if direction == 0 else xp_r
                qTf = xT_f if direction == 0 else yT_f  # finalize elementwise operand
                ocol = 0 if direction == 0 else D

                for g in range(NGRP):
                    if mode in ("qkexp", "qkonly"):
                        oaccs = None
                    elif PV_MERGE:
                        oacc_m = psum.tile([D + 1, GRP * CH], F32,
                                           tag="oacc", bufs=2,
                                           name=f"oacc_{direction}_{g}")
                        oaccs = [oacc_m[:, cl * CH : (cl + 1) * CH]
                                 for cl in range(GRP)]
                    else:
                        oaccs = [
                            psum.tile([D + 1, CH], F32, tag="oacc", bufs=4,
                                      name=f"oacc_{direction}_{g}_{cl}")
                            for cl in range(GRP)
                        ]
                    if pending is not None:
                        emit_finalize_head(pending)

                    # Software-pipelined emission: QK(i+1) is emitted BEFORE
                    # PV(i) so in PE program order the next logits tile is
                    # computed while ACT exps the current one -- the next
                    # exp's input is ready the moment the ACT engine frees.
                    c0 = g * GRP

                    def emit_qk(i):
                        qk = psum.tile([P, GRP * CH], F32, tag="qk",
                                       bufs=QK_SLOTS,
                                       name=f"qk_{direction}_{g}_{i}")
                        for j in range(GRP):
                            if QK_TP:
                                nc.tensor.matmul(
                                    qk[0:D, j * CH : (j + 1) * CH],
                                    kT2[0:D, i * P : i * P + D],
                                    qT2[0:D, (c0 + j) * CH : (c0 + j + 1) * CH],
                                    start=True, stop=True,
                                )
                                nc.tensor.matmul(
                                    qk[D : 2 * D, j * CH : (j + 1) * CH],
                                    kT2[D : 2 * D, i * P + D : (i + 1) * P],
                                    qT2[D : 2 * D,
                                        (c0 + j) * CH : (c0 + j + 1) * CH],
                                    start=True, stop=True,
                                )
                            else:
                                nc.tensor.matmul(
                                    qk[:, j * CH : (j + 1) * CH],
                                    kT[:, i * P : (i + 1) * P],
                                    qT[:, (c0 + j) * CH : (c0 + j + 1) * CH],
                                    start=True,
                                    stop=True,
                                )
                        return qk

                    def emit_pv(i, et):
                        if PV_MERGE:
                            nc.tensor.matmul(
                                oacc_m,
                                vp[:, i, :],
                                et[:, :],
                                start=(i == 0) or mode == "noacc",
                                stop=(i == NT - 1) or mode == "noacc",
                            )
                            return
                        for j in range(GRP):
                            nc.tensor.matmul(
                                oaccs[j],
                                vp[:, i, :],
                                et[:, j * CH : (j + 1) * CH],
                                start=(i == 0) or mode == "noacc",
                                stop=(i == NT - 1) or mode == "noacc",
                            )

                    # PV is emitted one iteration late so the PE stream is
                    # [... QK(i+1), QK(i+2), PV(i) ...]: the qk-slot release
                    # by exp(i) feeds QK(i+2) without a PV in between, so the
                    # ACT engine is paced only by its own throughput.
                    if mode == "qkonly":
                        for i in range(NT):
                            qk = emit_qk(i)
                        if g == NGRP - 1 and direction == 1:
                            dump = fin.tile([P, CH], F32, tag="dump")
                            nc.vector.tensor_copy(out=dump, in_=qk[:, 0:CH])
                            nc.sync.dma_start(
                                out=out[0:P, 0 : 2 * D],
                                in_=dump[:, 0 : 2 * D],
                            )
                        continue
                    if mode == "pvonly":
                        for i in range(NT):
                            emit_pv(i, etc)
                        pending = {
                            "oaccs": oaccs,
                            "qTf": qTf,
                            "g": g,
                            "direction": direction,
                            "out_ap": out[:, ocol : ocol + D],
                        }
                        continue
                    qk = emit_qk(0)
                    prev = None
                    for i in range(NT):
                        et = etp.tile([P, GRP * CH], ET_DT, tag="et")
                        nc.scalar.activation(out=et, in_=qk, func=EXP)
                        if i + 1 < NT:
                            qk = emit_qk(i + 1)
                        if mode == "qkexp":
                            if i == NT - 1:
                                dump = fin.tile([P, GRP * CH], F32, tag="dump")
                                nc.vector.tensor_copy(out=dump, in_=et)
                                nc.sync.dma_start(
                                    out=out[0:P, 0 : 2 * D],
                                    in_=dump[:, 0 : 2 * D],
                                )
                            continue
                        if prev is not None:
                            emit_pv(*prev)
                        prev = (i, et)
                        # Delay finalize chunks to mid-loop: by then the osb
                        # copy/mul/recip chain has finished, so the trp tiles
                        # occupy borrowed "oacc" slots only briefly.
                        if pending is not None and i % 6 == 5 and i // 6 < GRP:
                            emit_finalize_chunk(pending, i // 6)
                    if prev is not None:
                        emit_pv(*prev)
                    if mode == "nofin":
                        dump = fin.tile([D + 1, CH], F32, tag="dump")
                        nc.vector.tensor_copy(out=dump, in_=oaccs[0])
                        nc.sync.dma_start(
                            out=out[0 : D + 1, 0:D], in_=dump[:, 0:D]
                        )
                    if mode in ("full", "noacc"):
                        pending = {
                            "oaccs": oaccs,
                            "qTf": qTf,
                            "g": g,
                            "direction": direction,
                            "out_ap": out[:, ocol : ocol + D],
                        }
              if mode in ("full", "noacc", "pvonly"):
                # flush the last group's finalize
                emit_finalize_head(pending)
                for cl in range(GRP):
                    emit_finalize_chunk(pending, cl)

            if loop > 1:
                # Hardware loop for timing: one NEFF executes the body
                # `loop` times.  Back-edge costs ~2-6us per iteration
                # (all-engine barrier + IRAM refetch) -- ~1% bias.
                with tc.For_i(0, loop, 1):
                    emit_repeat()
            else:
                for _rep in range(repeat):
                    emit_repeat()
    nc.compile()
    return nc


def _make_in_maps(x: np.ndarray, y: np.ndarray) -> list:
    x = np.ascontiguousarray(np.asarray(x, dtype=np.float32))
    y = np.ascontiguousarray(np.asarray(y, dtype=np.float32))
    assert x.shape == (B, S, D) and y.shape == (B, S, D)
    ident = np.eye(P, dtype=np.float32)
    in_maps = []
    for b in range(B):
        xt = np.ascontiguousarray(x[b].T)
        yt = np.ascontiguousarray(y[b].T)
        in_maps.append(
            {
                "x": x[b],
                "y": y[b],
                "xT": xt.astype(np.float16) if QK_DT == FP16 else xt,
                "yT": yt.astype(np.float16) if QK_DT == FP16 else yt,
                "xTf": xt,
                "yTf": yt,
                "ident": ident,
            }
        )
    return in_maps


def kernel(x: np.ndarray, y: np.ndarray) -> np.ndarray:
    if "nc" not in _CACHE:
        _CACHE["nc"] = _build()
    nc = _CACHE["nc"]

    in_maps = _make_in_maps(x, y)
    res = run_bass_kernel_spmd(nc, in_maps, list(range(B))).results
    return np.stack([res[b]["out"] for b in range(B)], axis=0)



# revision 37
# speedup vs baseline: 2887.8118x; 2887.8118x over previous
"""BiModalAttention Trainium2 kernel.

Math (per batch b):
    S = x @ y.T                      [4096, 4096]
    a1 = softmax_rows(S)  @ y * x    [4096, 64]
    a2 = softmax_rows(S.T) @ x * y   [4096, 64]
    out = concat(a1, a2, axis=-1)    [4096, 128]

Sharding: data-parallel over batch, one batch per NeuronCore (8 cores).

Per-core algorithm (one direction; the other swaps x and y):
  The logits S never touch HBM.  Because |S| <= ~45 for randn inputs,
  exp(S) stays inside fp32 range, so softmax needs no max-subtraction:
  softmax(S)_st = exp(S_st) / sum_t exp(S_st).

  Pass over S^T tiles [t=128, s-chunk]:
    ST = yT_slice.T @ xT_chunk          (PE, f32r)
    E  = exp(ST)                        (ACT, writes f32r)
    acc[s-chunk] += [y | 1].T @ E       (PE, accumulate over all t)
  acc is [65, s-chunk]: rows 0..63 = (softmax_rows(S) @ y).T unnormalized,
  row 64 = the softmax row sums.  Finalize: PE-transpose acc, multiply by
  x and by reciprocal(row sums) per-partition, DMA to out.
"""

import numpy as np

import concourse.bass as bass
import concourse.mybir as mybir
import concourse.tile as tile
from concourse import bacc
from concourse.bass_utils import run_bass_kernel_spmd

B = 8
S = 4096
D = 64
P = 128
NT = S // P     # 32 row tiles
CH = 512        # psum chunk (one bank of fp32)
NCH = S // CH   # 8 chunks
HALF = NCH // 2  # 4 chunks per half-pass

F32 = mybir.dt.float32
F32R = mybir.dt.float32r
BF16 = mybir.dt.bfloat16
FP16 = mybir.dt.float16
EXP = mybir.ActivationFunctionType.Exp

# exp output / PV operand dtype.  The ACT engine is output-write-bound for
# fp32 (~1.36 ns/col measured); bf16 halves the write bytes and runs at
# ~0.57 ns/col.  PV matmul in bf16 is the same 1 cycle/row as f32r, and
# bf16's ~0.4% relative error on the (positive) exp values is well inside
# the 2e-2 tolerance.
ET_DT = BF16

# QK logits matmul dtype.  F32R streams at the PE roofline (~215ns/512-col
# MM measured, weight loads hidden); FP16 with K=64 is pathologically slow
# (~412ns/MM) so 2-byte QK operands are NOT a win here.
QK_DT = F32R

_CACHE = {}

# --- experiment knobs (module-level so harnesses can poke them) ---
QK_SLOTS = 2    # psum ring slots for qk tiles ([128, GRP*CH] = 2 banks each)
QK_TP = False   # emit QK as 2 concurrent 64x64 tile-position half-matmuls
PV_MERGE = False  # single 1024-col PV matmul instead of 2x 512
QK_WDT = None   # unused (fp16 QK path supersedes the bf16-weights idea)
GRP_KNOB = 2    # s-chunks per group (exp width = GRP*512 cols)
PIPE_V2 = False  # delayed-PV job pipeline (see emit_repeat_v2)
PIPE_V3 = False  # strict QK+exp / PV phase separation (implies PIPE_V2 path)
PV_LAG = 1      # iterations between exp(i) and PV(i) emission in the v1 loop


def _build(repeat: int = 1, mode: str = "full", loop: int = 1):
    nc = bacc.Bacc()
    x = nc.declare_dram_parameter("x", [S, D], F32, isOutput=False)
    y = nc.declare_dram_parameter("y", [S, D], F32, isOutput=False)
    xT = nc.declare_dram_parameter("xT", [D, S], QK_DT, isOutput=False)
    yT = nc.declare_dram_parameter("yT", [D, S], QK_DT, isOutput=False)
    xTf = nc.declare_dram_parameter("xTf", [D, S], F32, isOutput=False)
    yTf = nc.declare_dram_parameter("yTf", [D, S], F32, isOutput=False)
    ident = nc.declare_dram_parameter("ident", [P, P], F32, isOutput=False)
    out = nc.declare_dram_parameter("out", [S, 2 * D], F32, isOutput=True)

    with tile.TileContext(nc) as tc:
        with (
            tc.tile_pool(name="singles", bufs=1) as singles,
            tc.tile_pool(name="et", bufs=6) as etp,
            tc.tile_pool(name="fin", bufs=2) as fin,
            tc.tile_pool(name="psum", bufs=2, space="PSUM") as psum,
        ):
            # ---- load inputs ----
            # f32r matmul operands straight from DRAM (host passes the same
            # fp32 bytes; hardware rounds/interprets f32r on read).  The
            # first compute needs yT tile 0 and xT chunks 0..1, so the loads
            # are split with the critical pieces first on the sync HWDGE;
            # everything not needed immediately goes via gpsimd SWDGE.
            xT_r = singles.tile([D, S], QK_DT)
            yT_r = singles.tile([D, S], QK_DT)
            Q1 = 1024
            nc.sync.dma_start(out=xT_r[:, 0:Q1], in_=xT[:, 0:Q1])
            nc.sync.dma_start(out=yT_r[:, 0:Q1], in_=yT[:, 0:Q1])
            nc.sync.dma_start(out=yT_r[:, Q1:S], in_=yT[:, Q1:S])
            nc.sync.dma_start(out=xT_r[:, Q1:S], in_=xT[:, Q1:S])
            x_sb = singles.tile([P, NT, D], F32)
            y_sb = singles.tile([P, NT, D], F32)
            y_re = y[:, :].rearrange("(n p) d -> p n d", p=P)
            x_re = x[:, :].rearrange("(n p) d -> p n d", p=P)
            NT1 = 8
            nc.gpsimd.dma_start(out=y_sb[:, 0:NT1], in_=y_re[:, 0:NT1])
            nc.gpsimd.dma_start(out=x_sb[:, 0:NT1], in_=x_re[:, 0:NT1])
            nc.gpsimd.dma_start(out=y_sb[:, NT1:NT], in_=y_re[:, NT1:NT])
            nc.gpsimd.dma_start(out=x_sb[:, NT1:NT], in_=x_re[:, NT1:NT])
            # fp32 transposed copies for the finalize elementwise multiply
            xT_f = singles.tile([D, S], F32)
            yT_f = singles.tile([D, S], F32)
            nc.gpsimd.dma_start(out=xT_f, in_=xTf[:, :])
            nc.gpsimd.dma_start(out=yT_f, in_=yTf[:, :])
            id_sb = singles.tile([P, P], F32)
            nc.gpsimd.dma_start(out=id_sb, in_=ident[:, :])

            # Duplicated-across-halves operands for tile-position QK halves
            if QK_TP:
                xT2 = singles.tile([P, S], QK_DT)
                yT2 = singles.tile([P, S], QK_DT)
                nc.gpsimd.dma_start(out=xT2[0:D, :], in_=xT[:, :])
                nc.gpsimd.dma_start(out=xT2[D : 2 * D, :], in_=xT[:, :])
                nc.gpsimd.dma_start(out=yT2[0:D, :], in_=yT[:, :])
                nc.gpsimd.dma_start(out=yT2[D : 2 * D, :], in_=yT[:, :])

            # [V | 1] stationary operands for the PV matmul
            vp_r = singles.tile([P, NT, D + 1], ET_DT)
            xp_r = singles.tile([P, NT, D + 1], ET_DT)
            ones_f = singles.tile([P, NT, 1], F32)
            nc.vector.memset(ones_f, 1.0)
            if mode in ("pvonly", "qkpv"):
                etc = singles.tile([P, 2 * CH], ET_DT)
                nc.vector.memset(etc, 0.25)
            nc.vector.tensor_copy(out=vp_r[:, :, D : D + 1], in_=ones_f)
            nc.vector.tensor_copy(out=xp_r[:, :, D : D + 1], in_=ones_f)
            nc.vector.tensor_copy(out=vp_r[:, 0:NT1, 0:D], in_=y_sb[:, 0:NT1])
            nc.vector.tensor_copy(out=xp_r[:, 0:NT1, 0:D], in_=x_sb[:, 0:NT1])
            nc.vector.tensor_copy(out=vp_r[:, NT1:NT, 0:D], in_=y_sb[:, NT1:NT])
            nc.vector.tensor_copy(out=xp_r[:, NT1:NT, 0:D], in_=x_sb[:, NT1:NT])

            # The s axis is processed in groups of GRP chunks (one qk tile /
            # exp covers a whole group).  With GRP=2 the two live PSUM
            # accumulators of a group plus the two of the previous group fit
            # in 4 banks, so a new group's PV matmuls never wait on finalize.
            # Finalize work for a completed group is interleaved into the
            # NEXT group's loop: the oacc->sbuf copies are emitted up front
            # (freeing the accumulators), the transpose+scale+store of each
            # chunk spreads over the first iterations.  trp tiles borrow the
            # frequently-recycled "qk" slots (sharing "oacc" would deadlock
            # against the live accumulators of the current group).
            GRP = GRP_KNOB
            NGRP = NCH // GRP

            def emit_finalize_head(pending):
                osbs = []
                for cl in range(GRP):
                    c = pending["g"] * GRP + cl
                    osb = fin.tile([D + 1, CH], F32, tag="osb", bufs=4,
                                   name=f"osb_{pending['direction']}_{pending['g']}_{cl}")
                    nc.vector.tensor_copy(out=osb, in_=pending["oaccs"][cl])
                    osbs.append(osb)
                for cl in range(GRP):
                    c = pending["g"] * GRP + cl
                    nc.vector.tensor_mul(
                        osbs[cl][0:D, :], osbs[cl][0:D, :],
                        pending["qTf"][:, c * CH : (c + 1) * CH],
                    )
                    nc.vector.reciprocal(
                        out=osbs[cl][D : D + 1, :], in_=osbs[cl][D : D + 1, :]
                    )
                pending["osbs"] = osbs

            def emit_finalize_chunk(pending, cl, trp_tag="oacc", trp_bufs=4):
                c = pending["g"] * GRP + cl
                osb = pending["osbs"][cl]
                for k in range(CH // P):
                    st = c * (CH // P) + k  # s-tile index
                    trp = psum.tile([P, D + 1], F32, tag=trp_tag,
                                    bufs=trp_bufs)
                    nc.tensor.transpose(
                        trp,
                        osb[:, k * P : (k + 1) * P],
                        id_sb[: D + 1, : D + 1],
                    )
                    a_t = fin.tile([P, D], F32, tag="a", bufs=4)
                    nc.vector.tensor_scalar_mul(
                        a_t, trp[:, 0:D], trp[:, D : D + 1]
                    )
                    nc.sync.dma_start(
                        out=pending["out_ap"][st * P : (st + 1) * P],
                        in_=a_t,
                    )

            def emit_repeat_v2():
                # Delayed-PV pipeline: the work is 8 "jobs" (2 directions x
                # 4 chunk-groups).  Job k's QK+exp stream is interleaved
                # with job k-1's PV matmuls and job k-2's finalize.  PV
                # only ever reads exp outputs that completed a whole job
                # ago, so the in-order PE never parks on an exp-gated PV;
                # the only live chain is exp(k,i) -> qk-slot -> QK(k,i+3)
                # which 3 qk slots absorb.  et tiles buffer a full job
                # (NT+8 ring slots in SBUF).
                ET_BUFS = NT + 8
                jobs = []
                for direction in range(2):
                    for g in range(NGRP):
                        jobs.append({"direction": direction, "g": g})

                def job_ops(jb):
                    d = jb["direction"]
                    return (
                        xT_r if d == 0 else yT_r,   # qT (moving)
                        yT_r if d == 0 else xT_r,   # kT (stationary)
                        vp_r if d == 0 else xp_r,   # vp
                        xT_f if d == 0 else yT_f,   # qTf
                        0 if d == 0 else D,         # ocol
                    )

                def qk_of(jb, i):
                    qT, kT, _, _, _ = job_ops(jb)
                    c0 = jb["g"] * GRP
                    qk = psum.tile(
                        [P, GRP * CH], F32, tag="qk", bufs=QK_SLOTS,
                        name=f"qk_{jb['direction']}_{jb['g']}_{i}")
                    for j in range(GRP):
                        nc.tensor.matmul(
                            qk[:, j * CH : (j + 1) * CH],
                            kT[:, i * P : (i + 1) * P],
                            qT[:, (c0 + j) * CH : (c0 + j + 1) * CH],
                            start=True, stop=True,
                        )
                    return qk

                def pv_of(jb, i):
                    _, _, vp, _, _ = job_ops(jb)
                    for j in range(GRP):
                        nc.tensor.matmul(
                            jb["pending"]["oaccs"][j],
                            vp[:, i, :],
                            jb["ets"][i][:, j * CH : (j + 1) * CH],
                            start=(i == 0), stop=(i == NT - 1),
                        )
                    if i == NT - 1:
                        jb["ets"] = None  # release references

                def alloc_oaccs(jb):
                    _, _, _, qTf, ocol = job_ops(jb)
                    jb["pending"] = {
                        "oaccs": [
                            psum.tile(
                                [D + 1, CH], F32, tag="oacc", bufs=2,
                                name=f"oacc_{jb['direction']}_{jb['g']}_{cl}")
                            for cl in range(GRP)
                        ],
                        "qTf": qTf,
                        "g": jb["g"],
                        "direction": jb["direction"],
                        "out_ap": out[:, ocol : ocol + D],
                    }

                if PIPE_V3:
                    # Strict phase separation per job: the three concurrent
                    # psum streams (QK writes, ACT reads, PV accumulate RMW)
                    # measurably destroy each other's throughput, so never
                    # run them together.  Phase A: QK+exp i-loop at the
                    # ACT-saturated rate.  Phase B: PV matmuls back-to-back
                    # (450ns/iter), with the previous job's finalize
                    # interleaved (its trp scratch borrows the idle qk ring).
                    for k in range(len(jobs)):
                        jb = jobs[k]
                        jb["ets"] = []
                        qk = qk_of(jb, 0)
                        for i in range(NT):
                            et = etp.tile([P, GRP * CH], ET_DT, tag="et",
                                          bufs=ET_BUFS)
                            nc.scalar.activation(out=et, in_=qk, func=EXP)
                            jb["ets"].append(et)
                            if i + 1 < NT:
                                qk = qk_of(jb, i + 1)
                        alloc_oaccs(jb)
                        if k >= 1:
                            emit_finalize_head(jobs[k - 1]["pending"])
                        for i in range(NT):
                            pv_of(jb, i)
                            if k >= 1 and i % 6 == 5 and i // 6 < GRP:
                                emit_finalize_chunk(
                                    jobs[k - 1]["pending"], i // 6,
                                    trp_tag="qk", trp_bufs=QK_SLOTS)
                    emit_finalize_head(jobs[-1]["pending"])
                    for cl in range(GRP):
                        emit_finalize_chunk(jobs[-1]["pending"], cl,
                                            trp_tag="qk", trp_bufs=QK_SLOTS)
                    return

                for k in range(len(jobs) + 1):
                    jb = jobs[k] if k < len(jobs) else None
                    if k >= 1:
                        alloc_oaccs(jobs[k - 1])
                    if k >= 2:
                        emit_finalize_head(jobs[k - 2]["pending"])
                    if jb is not None:
                        jb["ets"] = []
                        qk = qk_of(jb, 0)
                        for i in range(NT):
                            et = etp.tile([P, GRP * CH], ET_DT, tag="et",
                                          bufs=ET_BUFS)
                            nc.scalar.activation(out=et, in_=qk, func=EXP)
                            jb["ets"].append(et)
                            if i + 1 < NT:
                                qk = qk_of(jb, i + 1)
                            if k >= 1:
                                pv_of(jobs[k - 1], i)
                            if k >= 2 and i % 6 == 5 and i // 6 < GRP:
                                emit_finalize_chunk(
                                    jobs[k - 2]["pending"], i // 6,
                                    trp_tag="qk", trp_bufs=QK_SLOTS)
                    else:
                        # epilogue: last job's PV + the last two finalizes
                        for i in range(NT):
                            pv_of(jobs[-1], i)
                            if i % 6 == 5 and i // 6 < GRP:
                                emit_finalize_chunk(
                                    jobs[-2]["pending"], i // 6,
                                    trp_tag="qk", trp_bufs=QK_SLOTS)
                        emit_finalize_head(jobs[-1]["pending"])
                        for cl in range(GRP):
                            emit_finalize_chunk(
                                jobs[-1]["pending"], cl,
                                trp_tag="qk", trp_bufs=QK_SLOTS)

            def emit_repeat():
              if (PIPE_V2 or PIPE_V3) and mode == "full":
                  emit_repeat_v2()
                  return
              pending = None
              for direction in range(2):
                qT = xT_r if direction == 0 else yT_r   # moving operand (s axis)
                kT = yT_r if direction == 0 else xT_r   # stationary (t axis)
                if QK_TP:
                    qT2 = xT2 if direction == 0 else yT2
                    kT2 = yT2 if direction == 0 else xT2
                vp = vp_r if direction == 0 else xp_r
                qTf = xT_f if direction == 0 else yT_f  # finalize elementwise operand
                ocol = 0 if direction == 0 else D

                for g in range(NGRP):
                    if mode in ("qkexp", "qkonly"):
                        oaccs = None
                    elif PV_MERGE:
                        oacc_m = psum.tile([D + 1, GRP * CH], F32,
                                           tag="oacc", bufs=2,
                                           name=f"oacc_{direction}_{g}")
                        oaccs = [oacc_m[:, cl * CH : (cl + 1) * CH]
                                 for cl in range(GRP)]
                    else:
                        oaccs = [
                            psum.tile([D + 1, CH], F32, tag="oacc", bufs=4,
                                      name=f"oacc_{direction}_{g}_{cl}")
                            for cl in range(GRP)
                        ]
                    if pending is not None:
                        emit_finalize_head(pending)

                    # Software-pipelined emission: QK(i+1) is emitted BEFORE
                    # PV(i) so in PE program order the next logits tile is
                    # computed while ACT exps the current one -- the next
                    # exp's input is ready the moment the ACT engine frees.
                    c0 = g * GRP

                    def emit_qk(i):
                        qk = psum.tile([P, GRP * CH], F32, tag="qk",
                                       bufs=QK_SLOTS,
                                       name=f"qk_{direction}_{g}_{i}")
                        for j in range(GRP):
                            if QK_TP:
                                nc.tensor.matmul(
                                    qk[0:D, j * CH : (j + 1) * CH],
                                    kT2[0:D, i * P : i * P + D],
                                    qT2[0:D, (c0 + j) * CH : (c0 + j + 1) * CH],
                                    start=True, stop=True,
                                )
                                nc.tensor.matmul(
                                    qk[D : 2 * D, j * CH : (j + 1) * CH],
                                    kT2[D : 2 * D, i * P + D : (i + 1) * P],
                                    qT2[D : 2 * D,
                                        (c0 + j) * CH : (c0 + j + 1) * CH],
                                    start=True, stop=True,
                                )
                            else:
                                nc.tensor.matmul(
                                    qk[:, j * CH : (j + 1) * CH],
                                    kT[:, i * P : (i + 1) * P],
                                    qT[:, (c0 + j) * CH : (c0 + j + 1) * CH],
                                    start=True,
                                    stop=True,
                                )
                        return qk

                    def emit_pv(i, et):
                        if PV_MERGE:
                            nc.tensor.matmul(
                                oacc_m,
                                vp[:, i, :],
                                et[:, :],
                                start=(i == 0) or mode == "noacc",
                                stop=(i == NT - 1) or mode == "noacc",
                            )
                            return
                        for j in range(GRP):
                            nc.tensor.matmul(
                                oaccs[j],
                                vp[:, i, :],
                                et[:, j * CH : (j + 1) * CH],
                                start=(i == 0) or mode == "noacc",
                                stop=(i == NT - 1) or mode == "noacc",
                            )

                    # PV is emitted one iteration late so the PE stream is
                    # [... QK(i+1), QK(i+2), PV(i) ...]: the qk-slot release
                    # by exp(i) feeds QK(i+2) without a PV in between, so the
                    # ACT engine is paced only by its own throughput.
                    if mode == "qkonly":
                        for i in range(NT):
                            qk = emit_qk(i)
                        if g == NGRP - 1 and direction == 1:
                            dump = fin.tile([P, CH], F32, tag="dump")
                            nc.vector.tensor_copy(out=dump, in_=qk[:, 0:CH])
                            nc.sync.dma_start(
                                out=out[0:P, 0 : 2 * D],
                                in_=dump[:, 0 : 2 * D],
                            )
                        continue
                    if mode == "qkpv":
                        for i in range(NT):
                            emit_qk(i)
                            emit_pv(i, etc)
                        pending = {
                            "oaccs": oaccs,
                            "qTf": qTf,
                            "g": g,
                            "direction": direction,
                            "out_ap": out[:, ocol : ocol + D],
                        }
                        continue
                    if mode == "pvonly":
                        for i in range(NT):
                            emit_pv(i, etc)
                        pending = {
                            "oaccs": oaccs,
                            "qTf": qTf,
                            "g": g,
                            "direction": direction,
                            "out_ap": out[:, ocol : ocol + D],
                        }
                        continue
                    qk = emit_qk(0)
                    pvq = []
                    for i in range(NT):
                        et = etp.tile([P, GRP * CH], ET_DT, tag="et")
                        nc.scalar.activation(out=et, in_=qk, func=EXP)
                        if i + 1 < NT:
                            qk = emit_qk(i + 1)
                        if mode == "qkexp":
                            if i == NT - 1:
                                dump = fin.tile([P, GRP * CH], F32, tag="dump")
                                nc.vector.tensor_copy(out=dump, in_=et)
                                nc.sync.dma_start(
                                    out=out[0:P, 0 : 2 * D],
                                    in_=dump[:, 0 : 2 * D],
                                )
                            continue
                        pvq.append((i, et))
                        if len(pvq) > PV_LAG:
                            emit_pv(*pvq.pop(0))
                        # Delay finalize chunks to mid-loop: by then the osb
                        # copy/mul/recip chain has finished, so the trp tiles
                        # occupy borrowed "oacc" slots only briefly.
                        if pending is not None and i % 6 == 5 and i // 6 < GRP:
                            emit_finalize_chunk(pending, i // 6)
                    for pv in pvq:
                        emit_pv(*pv)
                    if mode == "nofin":
                        dump = fin.tile([D + 1, CH], F32, tag="dump")
                        nc.vector.tensor_copy(out=dump, in_=oaccs[0])
                        nc.sync.dma_start(
                            out=out[0 : D + 1, 0:D], in_=dump[:, 0:D]
                        )
                    if mode in ("full", "noacc"):
                        pending = {
                            "oaccs": oaccs,
                            "qTf": qTf,
                            "g": g,
                            "direction": direction,
                            "out_ap": out[:, ocol : ocol + D],
                        }
              if mode in ("full", "noacc", "pvonly", "qkpv"):
                # flush the last group's finalize
                emit_finalize_head(pending)
                for cl in range(GRP):
                    emit_finalize_chunk(pending, cl)

            if loop > 1:
                # Hardware loop for timing: one NEFF executes the body
                # `loop` times.  Back-edge costs ~2-6us per iteration
                # (all-engine barrier + IRAM refetch) -- ~1% bias.
                with tc.For_i(0, loop, 1):
                    emit_repeat()
            else:
                for _rep in range(repeat):
                    emit_repeat()
    nc.compile()
    return nc


def _make_in_maps(x: np.ndarray, y: np.ndarray) -> list:
    x = np.ascontiguousarray(np.asarray(x, dtype=np.float32))
    y = np.ascontiguousarray(np.asarray(y, dtype=np.float32))
    assert x.shape == (B, S, D) and y.shape == (B, S, D)
    ident = np.eye(P, dtype=np.float32)
    in_maps = []
    for b in range(B):
        xt = np.ascontiguousarray(x[b].T)
        yt = np.ascontiguousarray(y[b].T)
        in_maps.append(
            {
                "x": x[b],
                "y": y[b],
                "xT": xt.astype(np.float16) if QK_DT == FP16 else xt,
                "yT": yt.astype(np.float16) if QK_DT == FP16 else yt,
                "xTf": xt,
                "yTf": yt,
                "ident": ident,
            }
        )
    return in_maps


def kernel(x: np.ndarray, y: np.ndarray) -> np.ndarray:
    if "nc" not in _CACHE:
        _CACHE["nc"] = _build()
    nc = _CACHE["nc"]

    in_maps = _make_in_maps(x, y)
    res = run_bass_kernel_spmd(nc, in_maps, list(range(B))).results
    return np.stack([res[b]["out"] for b in range(B)], axis=0)

